# revision 1
# baseline (speedup 1.0000x reference)
"""Trainium2 Bass kernel for BoundaryOperator SpMM (gnn_message_passing).

out[r, :] = sum over nnz (r, c, v): v * features[c, :]   (segment-sum of
gathered feature rows). 3M nnz, 500k output rows, 64 features, 8 cores.

Strategy (1D edge-parallel with output-row sharding, no collectives):
  - Host sorts nonzeros by output row and tiles the output into 128-row
    blocks. Block b needs k_b = ceil(nnz_b/128) "chunks" of 128 nnz.
  - Blocks are dealt to the 8 cores grouped by k so every core executes an
    IDENTICAL static chunk schedule (SPMD: one program, per-core data).
  - Each chunk's 128 feature rows are fetched with one indirect DMA (the
    HW consumes exactly one offset per partition). Chunks are grouped in
    batches of C_GATHER for the DVE work: one tensor_tensor premultiplies
    the gathered rows by the nnz values (broadcast AP), one tensor_tensor
    builds the batched selection matrix
    S[i, c, m] = (iota[m] == rowlocal[i, c]) against a broadcast iota.
    Per chunk, the TensorEngine accumulates S_c.T @ Gv_c into the block's
    [128, 64] PSUM tile.
  - PSUM is evicted via ScalarE to SBUF and DMA'd to a compact per-core
    output; the host scatters the blocks back to global row order.
"""

import sys

import numpy as np

# Toolchain import fallback: prefer whatever the environment already has on
# sys.path (the axon site), else the repo checkout.
if "/opt/trn_rl_repo" not in sys.path:
    sys.path.append("/opt/trn_rl_repo")

P = 128          # partitions / nnz per chunk / rows per output block
DF = 64          # feature dim
N_CORES = 8
C_GATHER = 32    # chunks per indirect-DMA gather / DVE batch
IDXT = 512       # chunks per index-stream tile (multiple of C_GATHER)

_prog_cache: dict = {}


# ---------------------------------------------------------------------------
# Host-side planning: sort by row, block, deal blocks to cores.
# ---------------------------------------------------------------------------

def _plan(rows, cols, vals, num_out, n_cores):
    nnz = rows.shape[0]
    order = np.argsort(rows, kind="stable")
    r = rows[order].astype(np.int64)
    c = cols[order].astype(np.int64)
    v = vals[order].astype(np.float32)

    # Variable-span output blocks: span <= P rows, cut where the block's nnz
    # count lands at (or just under) a multiple of P, so the last chunk of
    # each block carries minimal padding (fixed 128-aligned blocks waste
    # E[64] nnz per block; this wastes ~6).
    row_counts = np.bincount(r, minlength=num_out)
    cumz = np.concatenate([[0], np.cumsum(row_counts)])
    T = 116
    starts, spans, bcounts = [], [], []
    s = 0
    while s < num_out:
        e_max = min(s + P, num_out)
        q = cumz[s + 1:e_max + 1] - cumz[s]
        qm = q % P
        good = np.flatnonzero((qm == 0) | (qm >= T))
        if len(good) and qm[-1] != 0:
            e = s + int(good[-1]) + 1
        else:
            e = e_max
        starts.append(s)
        spans.append(e - s)
        bcounts.append(int(cumz[e] - cumz[s]))
        s = e
    starts = np.asarray(starts, np.int64)
    spans = np.asarray(spans, np.int64)
    counts = np.asarray(bcounts, np.int64)
    nblk = len(starts)
    k = np.maximum((counts + P - 1) // P, 1)  # chunks per block (>=1 so every
    #                                           block's rows get written)

    # Deal blocks to cores grouped by k, padding each k-group to a multiple
    # of n_cores with dummy blocks (-1) so all cores share one k-sequence.
    ks_desc = np.sort(np.unique(k))[::-1]
    seq_ks = []                                   # shared per-core k sequence
    core_slot_block = [[] for _ in range(n_cores)]  # per core: block id or -1
    for kk in ks_desc:
        blocks_k = np.where(k == kk)[0]
        pad = (-len(blocks_k)) % n_cores
        padded = np.concatenate([blocks_k, np.full(pad, -1, np.int64)])
        per = len(padded) // n_cores
        for i in range(n_cores):
            core_slot_block[i].extend(padded[i::n_cores].tolist())
        seq_ks.extend([int(kk)] * per)
    seq_ks = np.asarray(seq_ks, np.int64)
    nslot = len(seq_ks)
    chunk_off = np.concatenate([[0], np.cumsum(seq_ks)])
    nch = int(chunk_off[-1])
    nch_pad = -(-nch // C_GATHER) * C_GATHER

    # Map block -> (core, slot)
    core_of_block = np.full(nblk, -1, np.int64)
    slot_of_block = np.full(nblk, -1, np.int64)
    for i in range(n_cores):
        sb = np.asarray(core_slot_block[i], np.int64)
        real = sb >= 0
        core_of_block[sb[real]] = i
        slot_of_block[sb[real]] = np.where(real)[0]

    # Per-nnz placement: rank within block -> (chunk, lane)
    b = np.searchsorted(starts, r, side="right") - 1
    j = np.arange(nnz) - cumz[starts[b]]
    lane = j & (P - 1)
    chunk_idx = chunk_off[slot_of_block[b]] + (j >> 7)
    core_n = core_of_block[b]

    cols_arr = np.zeros((n_cores, P, nch_pad), np.int32)
    rowl_arr = np.full((n_cores, P, nch_pad), -1.0, np.float32)
    vals_arr = np.zeros((n_cores, P, nch_pad), np.float32)
    cols_arr[core_n, lane, chunk_idx] = c
    rowl_arr[core_n, lane, chunk_idx] = (r - starts[b]).astype(np.float32)
    vals_arr[core_n, lane, chunk_idx] = v

    # meta stream: per IDXT tile, [rowl_w | vals_w] so both live in one
    # DMA'd SBUF tile with contiguous per-chunk scalar runs.
    meta_arr = np.empty((n_cores, P, 2 * nch_pad), np.float32)
    for t in range(0, nch_pad, IDXT):
        w = min(IDXT, nch_pad - t)
        meta_arr[:, :, 2 * t:2 * t + w] = rowl_arr[:, :, t:t + w]
        meta_arr[:, :, 2 * t + w:2 * (t + w)] = vals_arr[:, :, t:t + w]

    plan = {
        "seq_ks": seq_ks,
        "nslot": nslot,
        "nch_pad": nch_pad,
        "core_slot_block": core_slot_block,
        "nblk": nblk,
        "starts": starts,
        "spans": spans,
    }
    return plan, cols_arr, meta_arr


def _reassemble(plan, results_key, results, num_out, n_cores):
    nslot = plan["nslot"]
    starts, spans = plan["starts"], plan["spans"]
    out = np.zeros((num_out, DF), np.float32)
    for i in range(n_cores):
        sb = np.asarray(plan["core_slot_block"][i], np.int64)
        res = np.asarray(results[i][results_key]).reshape(nslot, P, DF)
        for slot in range(nslot):
            blk = sb[slot]
            if blk >= 0:
                sp = spans[blk]
                out[starts[blk]:starts[blk] + sp] = res[slot, :sp]
    return out


# ---------------------------------------------------------------------------
# Bass program
# ---------------------------------------------------------------------------

def _split_excess_waits(nc, cap=1):
    """Hoist waits beyond `cap` per instruction onto standalone same-engine
    InstEventSemaphore instructions (walrus rejects >1 sync wait on many
    compute-instruction encodings)."""
    import concourse.mybir as mybir
    import bass_rust

    for bb in nc.main_func.blocks:
        new_insts = []
        for ins in bb.instructions:
            si = ins.sync_info
            tn = type(ins).__name__
            try:
                waits = list(si.on_wait)
            except Exception:
                waits = []
            if len(waits) > cap:
                keep = waits[:cap]
                for wt in waits[cap:]:
                    new_insts.append(mybir.InstEventSemaphore(
                        name=nc.get_next_instruction_name(),
                        engine=ins.engine,
                        ins=[],
                        outs=[],
                        sync_info=bass_rust.SyncInfo(
                            on_wait=[wt], on_update=[]),
                    ))
                ins.sync_info = bass_rust.SyncInfo(
                    on_wait=keep, on_update=list(si.on_update))
            new_insts.append(ins)
        bb.instructions = new_insts


def _build_program(num_e, nch, seq_ks, nslot):
    import concourse.bass as bass
    import concourse.mybir as mybir
    from concourse.tile import TileContext

    f32, i32 = mybir.dt.float32, mybir.dt.int32
    C = C_GATHER

    nc = bass.Bass()
    feat = nc.dram_tensor("features", [num_e, DF], f32, kind="ExternalInput")
    colsd = nc.dram_tensor("cols_arr", [P, nch], i32, kind="ExternalInput")
    metad = nc.dram_tensor("meta_arr", [P, 2 * nch], f32, kind="ExternalInput")
    outd = nc.dram_tensor("out_local", [nslot * P, DF], f32, kind="ExternalOutput")

    with TileContext(nc) as tc:
        with (
            tc.tile_pool(name="const", bufs=1) as cpool,
            tc.tile_pool(name="idx", bufs=2) as ipool,
            tc.tile_pool(name="g", bufs=3) as gpool,
            tc.tile_pool(name="gv", bufs=2) as gvpool,
            tc.tile_pool(name="s", bufs=2) as spool,
            tc.tile_pool(name="o", bufs=4) as opool,
            tc.tile_pool(name="psum", bufs=4, space="PSUM") as ppool,
        ):
            iota_i = cpool.tile([P, P], i32)
            nc.gpsimd.iota(iota_i[:], pattern=[[1, P]], base=0, channel_multiplier=0)
            iota_f = cpool.tile([P, P], f32)
            nc.vector.tensor_copy(iota_f[:], iota_i[:])
            # iota broadcast over the chunk dim: [128, C, 128] with step 0
            ia = iota_f[:]
            iota_b = bass.AP(ia.tensor, ia.offset, [ia.ap[0], [0, C], ia.ap[1]])

            ci = 0
            cols_t = meta_t = gv_t = s_t = None
            w_off = 0
            for slot in range(nslot):
                kk = int(seq_ks[slot])
                psum = ppool.tile([P, DF], f32)
                for cci in range(kk):
                    if ci % IDXT == 0:
                        w = min(IDXT, nch - ci)
                        w_off = w
                        cols_t = ipool.tile([P, IDXT], i32, tag="cols")
                        meta_t = ipool.tile([P, 2 * IDXT], f32, tag="meta")
                        nc.sync.dma_start(out=cols_t[:, :w], in_=colsd[:, ci:ci + w])
                        nc.sync.dma_start(out=meta_t[:, :2 * w],
                                          in_=metad[:, 2 * ci:2 * (ci + w)])
                    if ci % C == 0:
                        o = ci % IDXT
                        # One indirect DMA per chunk (HW consumes exactly one
                        # offset per partition); all C chunks of the batch
                        # land in slices of one tile so the DVE ops batch.
                        g_t = gpool.tile([P, C * DF], f32)
                        for cg in range(C):
                            nc.gpsimd.indirect_dma_start(
                                out=g_t[:, cg * DF:(cg + 1) * DF],
                                out_offset=None,
                                in_=feat[:, :],
                                in_offset=bass.IndirectOffsetOnAxis(
                                    ap=cols_t[:, o + cg:o + cg + 1], axis=0
                                ),
                            )
                        gv_t = gvpool.tile([P, C * DF], f32)
                        nc.vector.tensor_tensor(
                            out=gv_t[:].rearrange("p (c f) -> p c f", c=C),
                            in0=g_t[:].rearrange("p (c f) -> p c f", c=C),
                            in1=meta_t[:, w_off + o:w_off + o + C]
                                .to_broadcast([P, C, DF]),
                            op=mybir.AluOpType.mult,
                        )
                        s_t = spool.tile([P, C * P], f32)
                        nc.vector.tensor_tensor(
                            out=s_t[:].rearrange("p (c m) -> p c m", c=C),
                            in0=iota_b,
                            in1=meta_t[:, o:o + C].to_broadcast([P, C, P]),
                            op=mybir.AluOpType.is_equal,
                        )
                    cc = ci % C
                    nc.tensor.matmul(
                        out=psum[:],
                        lhsT=s_t[:, cc * P:(cc + 1) * P],
                        rhs=gv_t[:, cc * DF:(cc + 1) * DF],
                        start=(cci == 0),
                        stop=(cci == kk - 1),
                    )
                    ci += 1
                ot = opool.tile([P, DF], f32)
                nc.scalar.copy(out=ot[:], in_=psum[:])
                nc.sync.dma_start(out=outd[slot * P:(slot + 1) * P, :], in_=ot[:])
    return nc


def _get_program(num_e, nch, seq_ks, nslot):
    key = (num_e, nch, nslot, seq_ks.tobytes())
    if key not in _prog_cache:
        _prog_cache[key] = _build_program(num_e, nch, seq_ks, nslot)
    return _prog_cache[key]


# ---------------------------------------------------------------------------
# Entry point
# ---------------------------------------------------------------------------

def kernel(simplex_features, boundary_values, boundary_rows, boundary_cols,
           num_out, _trace=False):
    from concourse.bass_utils import run_bass_kernel_spmd

    num_out = int(num_out)
    feats = np.ascontiguousarray(np.asarray(simplex_features, np.float32))
    num_e = feats.shape[0]

    plan, cols_arr, meta_arr = _plan(
        np.asarray(boundary_rows), np.asarray(boundary_cols),
        np.asarray(boundary_values), num_out, N_CORES)

    nc = _get_program(num_e, plan["nch_pad"], plan["seq_ks"], plan["nslot"])
    if not getattr(nc, "_waits_split", False):
        _split_excess_waits(nc)
        nc._waits_split = True

    in_maps = [
        {
            "features": feats,
            "cols_arr": np.ascontiguousarray(cols_arr[i]),
            "meta_arr": np.ascontiguousarray(meta_arr[i]),
        }
        for i in range(N_CORES)
    ]
    res = run_bass_kernel_spmd(nc, in_maps, list(range(N_CORES)), trace=_trace)
    out = _reassemble(plan, "out_local", res.results, num_out, N_CORES)
    if _trace:
        return out, res
    return out


def estimate_core_time_ns(simplex_features, boundary_values, boundary_rows,
                          boundary_cols, num_out):
    """Cost-model span (ns) of one core's program via no-exec CoreSim."""
    from concourse.bass_interp import CoreSim

    num_out = int(num_out)
    plan, _, _ = _plan(
        np.asarray(boundary_rows), np.asarray(boundary_cols),
        np.asarray(boundary_values), num_out, N_CORES)
    nc = _build_program(np.asarray(simplex_features).shape[0],
                        plan["nch_pad"], plan["seq_ks"], plan["nslot"])
    sim = CoreSim(nc, no_exec=True, publish_trace=False)
    sim.simulate()
    return int(sim.time)



# revision 2
# speedup vs baseline: 1.1085x; 1.1085x over previous
"""Trainium2 Bass kernel for BoundaryOperator SpMM (gnn_message_passing), v2.

out[r, :] = sum over nnz (r, c, v): v * features[c, :].  3M nnz, 500k output
rows, 64 features, 8 cores.

Architecture (row-sharded, SWDGE gather/scatter, no matmul):
  - Core i owns output rows [i*62500, (i+1)*62500) and the ~375k nnz whose
    row lands there.  Output rows split into 2 windows of <=32768 rows so
    scatter indices fit int16.
  - Within a window, nnz are grouped into CELLS by 32768-row feature
    segment (col >> 15), so one `dma_gather` per cell fetches all feature
    rows with int16 indices against a per-cell segment base.
  - Gathered rows are multiplied by the nnz values on DVE (one batched
    tensor_tensor per cell).
  - Each cell's entries are split into occurrence-RANK groups (k-th entry
    of a row in the cell -> group k) so every `dma_scatter_add` has unique
    target rows; scatters are chained on a DMA-completion semaphore so no
    two scatters (which may share rows) are in flight together.
  - Scatter pads point at distinct unused rows with value 0 (adds 0.0).
  - All 8 cores share one instruction grid (per-cell/rank caps = max over
    cores), keeping the program SPMD; per-core index/value streams differ.

Cost model: SWDGE gather/scatter are charged free_size * CYCLE_T[Pool]
(~0.42ns/nnz each) on the Pool engine vs the 500ns-per-chunk floor of the
indirect-DMA path, with DVE/SP/Act work hidden underneath.
"""

import sys

import numpy as np

if "/opt/trn_rl_repo" not in sys.path:
    sys.path.append("/opt/trn_rl_repo")

P = 128
DF = 64
N_CORES = 8
WIN = 32768          # rows per scatter window (int16 index range)
SEG = 32768          # feature rows per gather segment

_prog_cache: dict = {}


# ---------------------------------------------------------------------------
# Host-side planning
# ---------------------------------------------------------------------------

def _plan(rows, cols, vals, num_out, num_e):
    rows = np.asarray(rows, np.int64)
    cols = np.asarray(cols, np.int64)
    vals = np.asarray(vals, np.float32)
    rows_per_core = num_out // N_CORES
    assert num_out % N_CORES == 0
    n_win = -(-rows_per_core // WIN)
    n_seg = -(-num_e // SEG)

    core = rows // rows_per_core
    lrow = rows - core * rows_per_core
    win = lrow >> 15
    wrow = lrow & (WIN - 1)
    seg = cols >> 15
    segcol = cols & (SEG - 1)

    # Per (core, win, seg) cell: entries with per-row occurrence rank.
    # Cells are split into sub-cells of <= CAPR raw entries to bound the
    # SWDGE descriptor-ring occupancy per instruction.
    CAPR = 10 ** 9   # no sub-cell split (single_packet=False handles big cells)
    order = np.lexsort((rows, seg, win, core))
    c_s, w_s, s_s, r_s = core[order], win[order], seg[order], rows[order]
    sc_s, wr_s, v_s = segcol[order], wrow[order], vals[order]
    idx_all = np.arange(len(order))
    # sub-cell = index within the (core, win, seg) group // CAPR
    cell_change = np.ones(len(order), bool)
    cell_change[1:] = (s_s[1:] != s_s[:-1]) | (w_s[1:] != w_s[:-1]) | \
                      (c_s[1:] != c_s[:-1])
    cell_id = np.cumsum(cell_change) - 1
    cell_starts = np.flatnonzero(cell_change)
    sub_s = (idx_all - cell_starts[cell_id]) // CAPR
    n_sub = int(sub_s.max()) + 1
    # fold sub into the segment key for grid purposes
    k_s = s_s * n_sub + sub_s
    n_key = n_seg * n_sub
    # run starts where (core, win, key, row) changes
    key_change = np.ones(len(order), bool)
    key_change[1:] = (r_s[1:] != r_s[:-1]) | (k_s[1:] != k_s[:-1]) | \
                     (w_s[1:] != w_s[:-1]) | (c_s[1:] != c_s[:-1])
    run_id = np.cumsum(key_change) - 1
    run_starts = np.flatnonzero(key_change)
    rank_s = idx_all - run_starts[run_id]

    # counts[core, win, key, rank]
    max_rank = int(rank_s.max()) + 1
    cnt = np.zeros((N_CORES, n_win, n_key, max_rank), np.int64)
    np.add.at(cnt, (c_s, w_s, k_s, rank_s), 1)

    # Common grid: per (win, key, rank) cap = max over cores, 128-aligned.
    caps = cnt.max(axis=0)                       # [n_win, n_key, max_rank]
    caps = ((caps + P - 1) // P) * P

    # Instruction tables (identical across cores).
    # Per window: list of cells (seg, pos_base, cell_cap, [(rank_cap, pos)]).
    windows = []
    pos = 0
    for w in range(n_win):
        cells = []
        for k in range(n_key):
            rlist = []
            base = pos
            for r in range(max_rank):
                cap = int(caps[w, k, r])
                if cap == 0:
                    break
                rlist.append((pos, cap))
                pos += cap
            if rlist:
                cells.append((k // n_sub, base, pos - base, rlist))
        windows.append(cells)
    totpos = pos
    assert totpos % P == 0

    # Per-core streams.
    gidx = np.zeros((N_CORES, totpos), np.int16)
    sidx = np.zeros((N_CORES, totpos), np.int16)
    valp = np.zeros((N_CORES, totpos), np.float32)

    # Position of each sorted nnz: pos_base(core,w,s,rank) + index-within.
    # Build per-(c,w,s,r) base table from grid caps.
    base_tab = np.zeros((n_win, n_key, max_rank), np.int64)
    pos2 = 0
    for w in range(n_win):
        for k in range(n_key):
            for r in range(max_rank):
                cap = int(caps[w, k, r])
                if cap == 0:
                    break
                base_tab[w, k, r] = pos2
                pos2 += cap
    # index within (c,w,s,r) group: entries are consecutive in sorted order
    # for fixed (c,w,s) with rank increasing within each row-run; regroup via
    # a second sort by (core, win, seg, rank).
    order2 = np.lexsort((rank_s, k_s, w_s, c_s))
    c2, w2, s2, r2 = c_s[order2], w_s[order2], k_s[order2], rank_s[order2]
    sc2, wr2, v2 = sc_s[order2], wr_s[order2], v_s[order2]
    grp_change = np.ones(len(order2), bool)
    grp_change[1:] = (r2[1:] != r2[:-1]) | (s2[1:] != s2[:-1]) | \
                     (w2[1:] != w2[:-1]) | (c2[1:] != c2[:-1])
    g_id = np.cumsum(grp_change) - 1
    g_starts = np.flatnonzero(grp_change)
    within = idx_all - g_starts[g_id]
    p2 = base_tab[w2, s2, r2] + within
    gidx[c2, p2] = sc2
    sidx[c2, p2] = wr2
    valp[c2, p2] = v2

    # Scatter pads: for each (core, win, seg, rank) group, pad slots get
    # distinct rows unused by that group (value stays 0 -> adds 0.0).
    filled = np.zeros((N_CORES, n_win, n_key, max_rank), np.int64)
    np.add.at(filled, (c2, w2, s2, r2), 1)
    for c in range(N_CORES):
        for w in range(n_win):
            for s in range(n_key):
                for r in range(max_rank):
                    cap = int(caps[w, s, r])
                    if cap == 0:
                        break
                    nfill = int(filled[c, w, s, r])
                    npad = cap - nfill
                    if npad == 0:
                        continue
                    b = base_tab[w, s, r]
                    used = sidx[c, b:b + nfill]
                    # candidate pad rows from the top of the window
                    cand = np.arange(WIN - 1, WIN - 1 - nfill - npad - 8, -1,
                                     dtype=np.int64)
                    cand = cand[~np.isin(cand, used)][:npad]
                    sidx[c, b + nfill:b + cap] = cand.astype(np.int16)

    # Wrap streams into device layouts.
    # idx: [128, totpos/16] int16, replicated per 16-partition group.
    g16 = gidx.reshape(N_CORES, totpos // 16, 16).transpose(0, 2, 1)
    s16 = sidx.reshape(N_CORES, totpos // 16, 16).transpose(0, 2, 1)
    gidx_arr = np.tile(g16, (1, 8, 1)).copy()       # [N, 128, totpos/16]
    sidx_arr = np.tile(s16, (1, 8, 1)).copy()
    vals_arr = valp.reshape(N_CORES, totpos // P, P).transpose(0, 2, 1).copy()

    plan = {
        "windows": windows,
        "totpos": totpos,
        "n_win": n_win,
        "rows_per_core": rows_per_core,
        "cellcap_max": max((cc for cells in windows for _, _, cc, _ in cells),
                           default=P),
        "win_pos": [  # per window: (pos_base, npos)
        ],
    }
    for w, cells in enumerate(windows):
        if cells:
            b0 = cells[0][1]
            b1 = cells[-1][1] + cells[-1][2]
            plan["win_pos"].append((b0, b1 - b0))
        else:
            plan["win_pos"].append((0, 0))
    return plan, gidx_arr, sidx_arr, vals_arr


# ---------------------------------------------------------------------------
# Bass program
# ---------------------------------------------------------------------------

def _build_program(num_e, plan):
    import concourse.bacc as bacc
    import concourse.bass as bass
    import concourse.mybir as mybir
    from concourse.tile import TileContext

    f32, i16 = mybir.dt.float32, mybir.dt.int16
    totpos = plan["totpos"]
    windows = plan["windows"]
    cellmax = plan["cellcap_max"]
    out_rows = 2 * WIN      # window 1 scatter range may reach 65535
    zero_rows = ((plan["rows_per_core"] + P - 1) // P) * P

    nc = bacc.Bacc(None, target_bir_lowering=False,
                   dynamic_dma_scratch_size=49152)
    feat = nc.dram_tensor("features", [num_e, DF], f32, kind="ExternalInput")
    gidxd = nc.dram_tensor("gidx", [P, totpos // 16], i16, kind="ExternalInput")
    sidxd = nc.dram_tensor("sidx", [P, totpos // 16], i16, kind="ExternalInput")
    valsd = nc.dram_tensor("vals", [P, totpos // P], f32, kind="ExternalInput")
    outw = [nc.dram_tensor(f"out_w{w}", [WIN, DF], f32, kind="ExternalOutput")
            for w in range(len(windows))]

    with TileContext(nc) as tc:
        with (
            tc.tile_pool(name="z", bufs=1) as zpool,
            tc.tile_pool(name="idx", bufs=1) as ipool,
            tc.tile_pool(name="g", bufs=3) as gpool,
            tc.tile_pool(name="gv", bufs=2) as gvpool,
        ):
            # Zero the real output rows (Act engine, big contiguous DMAs).
            # Per-window tensors: window 1 scatters only wait on window-1
            # zeroing, which overlaps with window-0 processing.
            z_t = zpool.tile([P, 2048], f32)
            nc.vector.memset(z_t[:], 0.0)
            zrem = zero_rows
            for w in range(len(windows)):
                zr = min(WIN, ((zrem + P - 1) // P) * P)
                if zr <= 0:
                    break
                zrem -= zr
                zc = (zr // P) * DF
                ov = outw[w].ap()[0:zr, :].rearrange("(p c) f -> p (c f)", p=P)
                for o in range(0, zc, 2048):
                    ww = min(2048, zc - o)
                    nc.scalar.dma_start(out=ov[:, o:o + ww], in_=z_t[:, :ww])

            scat_sem = nc.alloc_semaphore("scat_dma")
            n_scat = 0

            # Software pipeline: gather(cell i+1) is emitted before the
            # scatters of cell i so the Pool engine overlaps gathers with
            # scatter sem-waits.  Index/value streams are loaded per window
            # (SBUF cannot hold both windows' streams at once).
            all_cells = [(w, cell) for w, cells in enumerate(windows)
                         for cell in cells]
            win_of_cell = [w for w, _ in all_cells]
            wtile = {}

            def load_window(w):
                b0, npos = plan["win_pos"][w]
                gix_t = ipool.tile([P, npos // 16], i16, tag="gix")
                nc.sync.dma_start(out=gix_t[:],
                                  in_=gidxd[:, b0 // 16:(b0 + npos) // 16])
                six_t = ipool.tile([P, npos // 16], i16, tag="six")
                nc.sync.dma_start(out=six_t[:],
                                  in_=sidxd[:, b0 // 16:(b0 + npos) // 16])
                val_t = ipool.tile([P, npos // P], f32, tag="val")
                nc.sync.dma_start(out=val_t[:],
                                  in_=valsd[:, b0 // P:(b0 + npos) // P])
                wtile[w] = (b0, gix_t, six_t, val_t)

            gv_tiles = {}
            cum_scat = {}   # cell index -> total scatters emitted through it

            def emit_gather(ci):
                w, (s, base, cap, rlist) = all_cells[ci]
                wb, gix_t, six_t, val_t = wtile[w]
                base = base - wb
                # gv slot (bufs=2) is reused by cell ci from cell ci-2, whose
                # scatter DMAs read it asynchronously: gate the DVE multiply
                # on their completion.
                if ci >= 2:
                    nc.vector.wait_ge(scat_sem, 16 * cum_scat[ci - 2])
                ncol = cap // P
                g_t = gpool.tile([P, (cellmax // P) * DF], f32, tag="g")
                seg_lo = s * SEG
                seg_hi = min(seg_lo + SEG, num_e)
                nc.gpsimd.dma_gather(
                    out_ap=g_t[:, :ncol * DF].rearrange("p (c f) -> p c f", f=DF),
                    in_ap=feat[seg_lo:seg_hi, :],
                    idxs_ap=gix_t[:, base // 16:(base + cap) // 16],
                    num_idxs=cap,
                    num_idxs_reg=cap,
                    elem_size=DF,
                    single_packet=False,
                )
                gv_t = gvpool.tile([P, (cellmax // P) * DF], f32, tag="gv")
                nc.vector.tensor_tensor(
                    out=gv_t[:, :ncol * DF].rearrange("p (c f) -> p c f", f=DF),
                    in0=g_t[:, :ncol * DF].rearrange("p (c f) -> p c f", f=DF),
                    in1=val_t[:, base // P:(base + cap) // P]
                        .to_broadcast([P, ncol, DF]),
                    op=mybir.AluOpType.mult,
                )
                gv_tiles[ci] = gv_t

            def emit_scatters(ci):
                nonlocal n_scat
                w, (s, base, cap, rlist) = all_cells[ci]
                wb, gix_t, six_t, val_t = wtile[w]
                gv_t = gv_tiles.pop(ci)
                for (rpos, rcap) in rlist:
                    o = rpos - base
                    prev = n_scat
                    nc.gpsimd.dma_scatter_add(
                        out_ap=outw[w][:, :],
                        in_ap=gv_t[:, (o // P) * DF:((o + rcap) // P) * DF]
                            .rearrange("p (c f) -> p c f", f=DF),
                        idxs_ap=six_t[:, (rpos - wb) // 16:(rpos - wb + rcap) // 16],
                        num_idxs=rcap,
                        num_idxs_reg=rcap,
                        elem_size=DF,
                        single_packet=False,
                    ).then_inc(scat_sem, 16)._maybe_wait_ge(
                        (scat_sem, 16 * prev) if prev > 0 else None)
                    n_scat += 1
                cum_scat[ci] = n_scat

            for ci in range(len(all_cells)):
                w = win_of_cell[ci]
                if w not in wtile:
                    load_window(w)
                emit_gather(ci)
                if ci > 0:
                    emit_scatters(ci - 1)
            if all_cells:
                emit_scatters(len(all_cells) - 1)

    nc.compile()
    return nc


def _grid_key(plan):
    return (plan["totpos"],
            tuple((w, s, base, cap, tuple(rl))
                  for w, cells in enumerate(plan["windows"])
                  for (s, base, cap, rl) in cells))


def _get_program(num_e, plan):
    key = (num_e, _grid_key(plan))
    if key not in _prog_cache:
        _prog_cache[key] = _build_program(num_e, plan)
    return _prog_cache[key]


# ---------------------------------------------------------------------------
# Entry point
# ---------------------------------------------------------------------------

def kernel(simplex_features, boundary_values, boundary_rows, boundary_cols,
           num_out, _trace=False):
    from concourse.bass_utils import run_bass_kernel_spmd

    num_out = int(num_out)
    feats = np.ascontiguousarray(np.asarray(simplex_features, np.float32))
    num_e = feats.shape[0]

    plan, gidx_arr, sidx_arr, vals_arr = _plan(
        np.asarray(boundary_rows), np.asarray(boundary_cols),
        np.asarray(boundary_values), num_out, num_e)

    nc = _get_program(num_e, plan)

    in_maps = [
        {
            "features": feats,
            "gidx": np.ascontiguousarray(gidx_arr[i]),
            "sidx": np.ascontiguousarray(sidx_arr[i]),
            "vals": np.ascontiguousarray(vals_arr[i]),
        }
        for i in range(N_CORES)
    ]
    res = run_bass_kernel_spmd(nc, in_maps, list(range(N_CORES)), trace=_trace)

    rpc = plan["rows_per_core"]
    out = np.empty((num_out, DF), np.float32)
    for i in range(N_CORES):
        parts = []
        rem = rpc
        for w in range(plan["n_win"]):
            take = min(WIN, rem)
            parts.append(np.asarray(res.results[i][f"out_w{w}"])[:take])
            rem -= take
        out[i * rpc:(i + 1) * rpc] = np.concatenate(parts, axis=0)
    if _trace:
        return out, res
    return out


def estimate_core_time_ns(simplex_features, boundary_values, boundary_rows,
                          boundary_cols, num_out):
    """Cost-model span (ns) of one core's program via no-exec CoreSim."""
    from concourse.bass_interp import CoreSim

    num_out = int(num_out)
    num_e = np.asarray(simplex_features).shape[0]
    plan, _, _, _ = _plan(
        np.asarray(boundary_rows), np.asarray(boundary_cols),
        np.asarray(boundary_values), num_out, num_e)
    nc = _get_program(num_e, plan)
    sim = CoreSim(nc, no_exec=True, publish_trace=False)
    sim.simulate()
    return int(sim.time)
